# revision 1
# baseline (speedup 1.0000x reference)
"""3D Haar DWT (2x2x2 blocks, 8 subbands) on 8 Trainium2 NeuronCores.

Input  x: (2, 16, 64, 128, 128) f32.
Output: tuple of 8 subbands, each (2, 16, 32, 64, 64) f32, subband order
LLL,LLH,LHL,LHH,HLL,HLH,HHL,HHH (filters applied to (D,H,W) resp.).

Strategy (pure data parallel, zero cross-core communication):
  - Flatten (B,C) -> 32 independent slabs of (64,128,128); core i takes 4.
  - Per slab: one 4MB DMA into an SBUF tile laid out as
      partitions = (q, d)           q = h parity (2), d = depth (64)
      free       = (hh, hb, w)      hh = h'-half (2), hb = h' sub (32), w (128)
    so that a single constant 128x128 matrix applied on the partition axis by
    the TensorEngine performs BOTH the D-axis and H-axis Haar butterflies:
      out partition m = (alpha, beta, d')   alpha=D band, beta=H band, d' (32)
      M[q*64 + 2d' + p, alpha*64 + beta*32 + d'] = f_alpha[p]*f_beta[q]*s
    (4 nonzeros per column; full 1/(2*sqrt2) scale folded in).
  - The W-axis butterfly (even +/- odd along w) runs on the VectorEngine with
    stride-2 access patterns, PSUM -> SBUF.
  - Each subband's slab result is one contiguous 512KB DMA to HBM.
"""

import numpy as np

_B, _C, _D, _H, _W = 2, 16, 64, 128, 128
_NCORES = 8
_SLABS = _B * _C  # 32
_SLABS_PER_CORE = _SLABS // _NCORES  # 4


def _haar_filters_np():
    # Bit-identical construction to the reference filter bank.
    s = 1.0 / np.sqrt(2.0)
    L = np.array([s, s], dtype=np.float32)
    H = np.array([s, -s], dtype=np.float32)
    bands = [(a, b, c) for a in "LH" for b in "LH" for c in "LH"]
    filt = np.stack(
        [
            (L if a == "L" else H)[:, None, None]
            * (L if b == "L" else H)[None, :, None]
            * (L if c == "L" else H)[None, None, :]
            for (a, b, c) in bands
        ],
        axis=0,
    )  # (8, 2, 2, 2) float32
    return filt


def _haar_matrix():
    """(128,128) f32 for the D-axis butterfly on the partition axis.

    Input partition  = d*2 + hh         (hh = h-half, d = depth 0..63)
    Output partition = a*64 + d'*2 + hh (a = D band, d' = 0..31)
    (d-major order keeps the DMA access patterns' outer dims large, so the
    HWDGE sprays transfers across all 16 SDMA engines.)
    Entry = f_a[p] * s * s  (d = 2d'+p), i.e. the reference filter value
    filt[a*4, p, 0, 0] — the full 1/(2*sqrt2) magnitude is folded here so
    the H/W butterflies on DVE are pure +/- adds."""
    filt = _haar_filters_np()
    M = np.zeros((128, 128), dtype=np.float32)
    for hh in range(2):
        for a in range(2):
            for dp in range(32):
                for p in range(2):
                    M[(2 * dp + p) * 2 + hh, a * 64 + dp * 2 + hh] = filt[
                        a * 4, p, 0, 0
                    ]
    return M


def _build_bass():
    import concourse.mybir as mybir
    import concourse.tile as tile
    from concourse import bacc

    f32 = mybir.dt.float32
    nc = bacc.Bacc("TRN2", target_bir_lowering=False, debug=False)

    x = nc.dram_tensor("x", [_SLABS_PER_CORE, _D, _H, _W], f32, kind="ExternalInput")
    hm = nc.dram_tensor("hm", [128, 128], f32, kind="ExternalInput")
    y = nc.dram_tensor(
        "y", [8, _SLABS_PER_CORE, _D // 2, _H // 2, _W // 2], f32, kind="ExternalOutput"
    )

    # x[t, d, h, w] with h = hh*64 + hb*2 + q.
    # SBUF layout: partitions (d, hh) [p = d*2+hh], free (hb, q, w) -- each
    # partition's free dim walks a CONTIGUOUS 32KB HBM region (one
    # descriptor per partition), and the DRAM-side AP's outer dim is d:64,
    # which the HWDGE deals round-robin across all 16 SDMA engines.
    xr = x[:, :, :, :].rearrange("t d (hh hb q) w -> t d hh hb q w", hh=2, hb=32, q=2)
    # y[s, t, dp, h', w'] with h' = hh*32 + hb; partition order (dp, hh).
    yr = y[:, :, :, :, :].rearrange("s t dp (hh hb) wp -> s t dp hh hb wp", hh=2)

    with tile.TileContext(nc) as tc:
        with (
            tc.tile_pool(name="const", bufs=1) as cpool,
            tc.tile_pool(name="xin", bufs=4) as xpool,
            tc.tile_pool(name="uband", bufs=1) as upool,
            tc.tile_pool(name="outs", bufs=2) as opool,
            tc.tile_pool(name="stage", bufs=2) as spool,
            tc.tile_pool(name="psum", bufs=2, space="PSUM") as ppool,
        ):
            hmt = cpool.tile([128, 128], f32, tag="hm")
            nc.sync.dma_start(out=hmt[:, :], in_=hm[:, :])

            def load_slab(t):
                # Whole slab: partitions (d, hh), free (hb, q, w) = 8192.
                # Four quarter-DMAs along the free dim (hb ranges) so the
                # first matmul chunk can start as soon as its quarter lands.
                # Issue inputs via the GPSIMD SWDGE queue: it is otherwise
                # idle and has its own ring, so input issue never queues
                # behind output DMAs (SP ring) or PSUM-drain copies (ACT),
                # which caused PE stalls / deadlocks on the HWDGE rings.
                # Two half-slab tiles (bufs=4) so a prefetched load only
                # waits on the matmuls of the same half two slabs back.
                halves = []
                for h in range(2):
                    xh = xpool.tile([128, 4096], f32, tag="xt", name=f"xt_{t}_{h}")
                    for c in range(2):
                        nc.gpsimd.dma_start(
                            out=xh[:, c * 2048 : (c + 1) * 2048],
                            in_=xr[t, :, :, (h * 2 + c) * 8 : (h * 2 + c + 1) * 8],
                        )
                    halves.append(xh)
                return halves

            # Software prefetch one slab ahead so next-slab input issue
            # front-runs this slab's PSUM-drain copies on the ACT ring.
            xt_next = load_slab(0)
            for t in range(_SLABS_PER_CORE):
                xt = xt_next
                if t + 1 < _SLABS_PER_CORE:
                    xt_next = load_slab(t + 1)

                # H-band intermediates (post D+H): free (hb, w) = 4096.
                # Written and read only by DVE -> bufs=1 is race-free.
                u = [
                    upool.tile([128, 4096], f32, tag=f"u{b}", name=f"u{b}_{t}")
                    for b in range(2)
                ]
                # Final subband tiles [beta][gamma]: free (hb, w') = 2048.
                o = [
                    [
                        opool.tile(
                            [128, 2048], f32, tag=f"o{b}{g}", name=f"o{b}{g}_{t}"
                        )
                        for g in range(2)
                    ]
                    for b in range(2)
                ]

                for c in range(4):  # 2048-wide chunks: hb in [8c, 8c+8)
                    pt = ppool.tile([128, 2048], f32, tag="pt")
                    xth = xt[c // 2]
                    for j in range(4):  # N=512 matmuls (fp32 moving-max)
                        lo = (c % 2) * 2048 + j * 512
                        nc.tensor.matmul(
                            pt[:, j * 512 : (j + 1) * 512],
                            hmt[:, :],
                            xth[:, lo : lo + 512],
                            start=True,
                            stop=True,
                        )
                    # DVE can't read two PSUM operands; stage via ScalarE
                    # (otherwise idle).
                    ct = spool.tile([128, 2048], f32, tag="ct")
                    nc.scalar.copy(ct[:, :], pt[:, :])
                    # H butterfly on DVE: pair q=0/q=1 (stride 256 blocks).
                    cr = ct[:, :].rearrange("m (hb q w) -> m hb q w", hb=8, q=2)
                    ev, od = cr[:, :, 0, :], cr[:, :, 1, :]
                    u0s = u[0][:, c * 1024 : (c + 1) * 1024].rearrange(
                        "m (hb w) -> m hb w", hb=8
                    )
                    u1s = u[1][:, c * 1024 : (c + 1) * 1024].rearrange(
                        "m (hb w) -> m hb w", hb=8
                    )
                    nc.vector.tensor_add(u0s, ev, od)
                    nc.vector.tensor_sub(u1s, ev, od)

                # W butterfly on DVE: even/odd w (stride 2).
                for b in range(2):
                    ur = u[b][:, :].rearrange("m (hb w2 r) -> m hb w2 r", hb=32, r=2)
                    ev, od = ur[:, :, :, 0], ur[:, :, :, 1]
                    o0 = o[b][0][:, :].rearrange("m (hb w2) -> m hb w2", hb=32)
                    o1 = o[b][1][:, :].rearrange("m (hb w2) -> m hb w2", hb=32)
                    nc.vector.tensor_add(o0, ev, od)
                    nc.vector.tensor_sub(o1, ev, od)

                last = t == _SLABS_PER_CORE - 1
                for s in range(8):
                    a, b, g = (s >> 2) & 1, (s >> 1) & 1, s & 1
                    # Partitions a*64 + (dp, hh) interleaved: one contiguous
                    # 64-partition block per subband.  Issue on SP so the ACT
                    # ring stays free to drain PSUM without delay; for the
                    # final slab (no more inputs/copies pending) split across
                    # both rings to halve the tail.
                    if last:
                        eng = (nc.sync, nc.scalar, nc.gpsimd)[s % 3]
                    else:
                        eng = nc.sync
                    eng.dma_start(
                        out=yr[s, t],
                        in_=o[b][g][a * 64 : (a + 1) * 64, :],
                    )
    nc.compile()
    return nc


_NC_CACHE = None


def _get_nc():
    global _NC_CACHE
    if _NC_CACHE is None:
        _NC_CACHE = _build_bass()
    return _NC_CACHE


def _run(x, trace=False, **spmd_kwargs):
    from concourse.bass_utils import run_bass_kernel_spmd

    x = np.ascontiguousarray(x, dtype=np.float32)
    xf = x.reshape(_SLABS, _D, _H, _W)
    M = _haar_matrix()
    in_maps = [
        {
            "x": np.ascontiguousarray(
                xf[i * _SLABS_PER_CORE : (i + 1) * _SLABS_PER_CORE]
            ),
            "hm": M,
        }
        for i in range(_NCORES)
    ]
    res = run_bass_kernel_spmd(
        _get_nc(), in_maps, core_ids=list(range(_NCORES)), trace=trace, **spmd_kwargs
    )
    outs = [r["y"] for r in res.results]  # each (8, 4, 32, 64, 64)
    full = np.concatenate(outs, axis=1)  # (8, 32, 32, 64, 64)
    full = full.reshape(8, _B, _C, _D // 2, _H // 2, _W // 2)
    return full, res


def kernel(**inputs):
    full, _ = _run(inputs["x"])
    return tuple(full[i] for i in range(8))



# revision 4
# speedup vs baseline: 1.1453x; 1.1453x over previous
"""3D Haar DWT (2x2x2 blocks, 8 subbands) on 8 Trainium2 NeuronCores.

Input  x: (2, 16, 64, 128, 128) f32.
Output: tuple of 8 subbands, each (2, 16, 32, 64, 64) f32, subband order
LLL,LLH,LHL,LHH,HLL,HLH,HHL,HHH (filters applied to (D,H,W) resp.).

Strategy (pure data parallel, zero cross-core communication), v2 = bf16:
  - The 2e-2 rel-err budget admits bf16 I/O (measured ~2e-3), halving HBM
    traffic per core to 8 MiB in + 8 MiB out -> ~47us DMA roofline/core.
  - Flatten (B,C) -> 32 slabs of (64,128,128); core i takes 4 = 2 PAIRS.
    A pair of slabs is processed together with SBUF partitions = (d, tp)
    (tp = slab parity).  Compared to (d, h-half) partitions this keeps
    per-partition DRAM runs at 16KB on input and 8KB on output (the h
    dimension stays whole in the free dim), so every DMA descriptor is
    large.
  - TensorEngine: one constant 128x128 matrix does the D-axis butterfly on
    the partition axis: M[(2dp+p)*2+tp, a*64+dp*2+tp] = f_a[p]/2 (full
    1/(2*sqrt2) scale folded in).  The matmul's moving-operand access
    pattern feeds columns in (hb, q, r, w2) order, so PSUM lands with the
    W-axis parity r DEINTERLEAVED -- both later butterflies then read/write
    unit-stride bf16 and hit the DVE 2x packed mode.
  - ScalarE drains PSUM f32 -> SBUF bf16 (cast on copy).
  - DVE: H butterfly (pair q blocks, stride-1) then W butterfly (pair r
    blocks, stride-1), all bf16 2x mode.
  - Input DMAs on SP (HWDGE), output DMAs on GPSIMD (SWDGE): each ring has
    one producer so issues never queue behind each other.
"""

import numpy as np
import ml_dtypes

_B, _C, _D, _H, _W = 2, 16, 64, 128, 128
_NCORES = 8
_SLABS = _B * _C  # 32
_SLABS_PER_CORE = _SLABS // _NCORES  # 4
_PAIRS = _SLABS_PER_CORE // 2  # 2

_BF16 = ml_dtypes.bfloat16


def _haar_filters_np():
    # Bit-identical construction to the reference filter bank.
    s = 1.0 / np.sqrt(2.0)
    L = np.array([s, s], dtype=np.float32)
    H = np.array([s, -s], dtype=np.float32)
    bands = [(a, b, c) for a in "LH" for b in "LH" for c in "LH"]
    filt = np.stack(
        [
            (L if a == "L" else H)[:, None, None]
            * (L if b == "L" else H)[None, :, None]
            * (L if c == "L" else H)[None, None, :]
            for (a, b, c) in bands
        ],
        axis=0,
    )  # (8, 2, 2, 2) float32
    return filt


def _haar_matrix():
    """(128,128) for the D-axis butterfly on the partition axis.

    Input partition  = d*2 + tp         (tp = slab parity, d = depth 0..63)
    Output partition = a*64 + tp*32 + dp (a = D band, dp = 0..31)
    (tp-major over dp so the output DMA's DRAM-side (tp, dp) dims merge:
    tp stride = 32 x dp stride -> 3-dim AP.)
    Entry = f_a[p] * s * s  (d = 2dp+p): the full 1/(2*sqrt2) magnitude is
    folded here so the H/W butterflies on DVE are pure +/- adds."""
    filt = _haar_filters_np()
    M = np.zeros((128, 128), dtype=np.float32)
    for tp in range(2):
        for a in range(2):
            for dp in range(32):
                for p in range(2):
                    M[(2 * dp + p) * 2 + tp, a * 64 + tp * 32 + dp] = filt[
                        a * 4, p, 0, 0
                    ]
    return M


def _build_bass():
    import concourse.mybir as mybir
    import concourse.tile as tile
    from concourse import bacc

    f32 = mybir.dt.float32
    bf16 = mybir.dt.bfloat16
    nc = bacc.Bacc("TRN2", target_bir_lowering=False, debug=False)

    x = nc.dram_tensor("x", [_SLABS_PER_CORE, _D, _H, _W], bf16, kind="ExternalInput")
    hm = nc.dram_tensor("hm", [128, 128], bf16, kind="ExternalInput")
    y = nc.dram_tensor(
        "y", [8, _SLABS_PER_CORE, _D // 2, _H // 2, _W // 2], bf16,
        kind="ExternalOutput",
    )

    # x[t=2pr+tp, d, h, w] with h = hh*64 + hb*2 + q, w = w2*2 + r.
    # Half-pair tile (pr, hh): partitions (d, tp), free (hb, q, w) -- each
    # partition's free dim walks a CONTIGUOUS 16KB HBM region; split into two
    # 1MB DMAs (hb halves, 8KB/partition each) so matmuls start early.
    xr = x[:, :, :, :].rearrange(
        "(pr tp) d (hh hb q) w -> pr d tp hh hb q w", tp=2, hh=2, hb=32, q=2
    )
    # y[s=(a,b,g), t=2pr+tp, dp, hp, wp]; partition order (a, tp, dp); each
    # partition's (hp, wp) run is 8KB contiguous in HBM.
    yv = y[:, :, :, :, :].rearrange(
        "(a b g) (pr tp) dp hp wp -> a b g pr tp dp hp wp", a=2, b=2, tp=2
    )

    with tile.TileContext(nc) as tc:
        with (
            tc.tile_pool(name="const", bufs=1) as cpool,
            tc.tile_pool(name="xin", bufs=3) as xpool,
            tc.tile_pool(name="stage", bufs=2) as spool,
            tc.tile_pool(name="uband", bufs=1) as upool,
            tc.tile_pool(name="outs", bufs=2) as opool,
            tc.tile_pool(name="psum", bufs=2, space="PSUM") as ppool,
        ):
            hmt = cpool.tile([128, 128], bf16, tag="hm")
            nc.sync.dma_start(out=hmt[:, :], in_=hm[:, :])

            def load_half_pair(pr, hh):
                xh = xpool.tile([128, 8192], bf16, tag="xt", name=f"xt_{pr}_{hh}")
                for c in range(2):
                    nc.sync.dma_start(
                        out=xh[:, c * 4096 : (c + 1) * 4096],
                        in_=xr[pr, :, :, hh, c * 16 : (c + 1) * 16],
                    )
                return xh

            HP = [(pr, hh) for pr in range(_PAIRS) for hh in range(2)]
            tiles = {HP[0]: load_half_pair(*HP[0]), HP[1]: load_half_pair(*HP[1])}

            # o tiles live at pair scope: [128, (hp, w2)] -- each half-pair's
            # W butterfly fills one hh half; one 1MB DMA per (b, g) drains
            # the whole pair with 8KB descriptors.
            o = None
            for k, (pr, hh) in enumerate(HP):
                if k + 2 < len(HP):
                    tiles[HP[k + 2]] = load_half_pair(*HP[k + 2])
                xt = tiles.pop((pr, hh))
                if hh == 0:
                    o = [
                        [
                            opool.tile(
                                [128, 4096], bf16, tag=f"o{b}{g}", name=f"o{b}{g}_{pr}"
                            )
                            for g in range(2)
                        ]
                        for b in range(2)
                    ]

                # Moving-operand AP in deinterleaved (hb, q, r, w2) order:
                # PSUM layout = (hb, q, r, w2).
                xv = xt[:, :].rearrange(
                    "m (hb q w2 r) -> m hb q r w2", hb=32, q=2, r=2
                )
                st = spool.tile([128, 8192], bf16, tag="st", name=f"st_{pr}_{hh}")
                for c in range(4):  # 2048-wide chunks: hb in [8c, 8c+8)
                    pt = ppool.tile([128, 2048], f32, tag="pt")
                    for j in range(4):  # N=512 matmuls (one PSUM bank each)
                        nc.tensor.matmul(
                            pt[:, j * 512 : (j + 1) * 512],
                            hmt[:, :],
                            xv[:, c * 8 + 2 * j : c * 8 + 2 * j + 2],
                            start=True,
                            stop=True,
                        )
                    # PSUM f32 -> SBUF bf16 (ScalarE, otherwise idle).
                    nc.scalar.copy(st[:, c * 2048 : (c + 1) * 2048], pt[:, :])

                # H butterfly on DVE: pair q=0/q=1 blocks of (r, w2)=128.
                sr = st[:, :].rearrange("m (hb q rw) -> m hb q rw", hb=32, q=2)
                ev, od = sr[:, :, 0, :], sr[:, :, 1, :]
                u = [
                    upool.tile([128, 4096], bf16, tag=f"u{b}", name=f"u{b}_{pr}_{hh}")
                    for b in range(2)
                ]
                u0s = u[0][:, :].rearrange("m (hb rw) -> m hb rw", hb=32)
                u1s = u[1][:, :].rearrange("m (hb rw) -> m hb rw", hb=32)
                nc.vector.tensor_add(u0s, ev, od)
                nc.vector.tensor_sub(u1s, ev, od)

                # W butterfly on DVE: pair r=0/r=1 blocks of w2=64.
                for b in range(2):
                    ur = u[b][:, :].rearrange("m (hb r w2) -> m hb r w2", hb=32, r=2)
                    uev, uod = ur[:, :, 0, :], ur[:, :, 1, :]
                    lo = hh * 2048
                    o0 = o[b][0][:, lo : lo + 2048].rearrange(
                        "m (hb w2) -> m hb w2", hb=32
                    )
                    o1 = o[b][1][:, lo : lo + 2048].rearrange(
                        "m (hb w2) -> m hb w2", hb=32
                    )
                    nc.vector.tensor_add(o0, uev, uod)
                    nc.vector.tensor_sub(o1, uev, uod)

                if hh == 1:
                    last = pr == _PAIRS - 1
                    for b in range(2):
                        for g in range(2):
                            # SWDGE ring is otherwise idle; on the final pair
                            # split across rings to halve the drain tail.
                            if last:
                                eng = (nc.gpsimd, nc.sync)[g]
                            else:
                                eng = nc.gpsimd
                            eng.dma_start(out=yv[:, b, g, pr], in_=o[b][g][:, :])
    nc.compile()
    return nc


_NC_CACHE = None


def _get_nc():
    global _NC_CACHE
    if _NC_CACHE is None:
        _NC_CACHE = _build_bass()
    return _NC_CACHE


def _run(x, trace=False, **spmd_kwargs):
    from concourse.bass_utils import run_bass_kernel_spmd

    x = np.ascontiguousarray(x, dtype=np.float32)
    xf = x.reshape(_SLABS, _D, _H, _W).astype(_BF16)
    M = _haar_matrix().astype(_BF16)
    in_maps = [
        {
            "x": np.ascontiguousarray(
                xf[i * _SLABS_PER_CORE : (i + 1) * _SLABS_PER_CORE]
            ),
            "hm": M,
        }
        for i in range(_NCORES)
    ]
    res = run_bass_kernel_spmd(
        _get_nc(), in_maps, core_ids=list(range(_NCORES)), trace=trace, **spmd_kwargs
    )
    outs = [r["y"] for r in res.results]  # each (8, 4, 32, 64, 64) bf16
    full = np.concatenate(outs, axis=1).astype(np.float32)  # (8, 32, 32, 64, 64)
    full = full.reshape(8, _B, _C, _D // 2, _H // 2, _W // 2)
    return full, res


def kernel(**inputs):
    full, _ = _run(inputs["x"])
    return tuple(full[i] for i in range(8))


# revision 7
# speedup vs baseline: 1.8336x; 1.6010x over previous
"""3D Haar DWT (2x2x2 blocks, 8 subbands) on 8 Trainium2 NeuronCores.

Input  x: (2, 16, 64, 128, 128) f32.
Output: tuple of 8 subbands, each (2, 16, 32, 64, 64) f32, subband order
LLL,LLH,LHL,LHH,HLL,HLH,HHL,HHH (filters applied to (D,H,W) resp.).

Strategy (pure data parallel, zero cross-core communication), v2 = bf16:
  - The 2e-2 rel-err budget admits bf16 I/O (measured ~2e-3), halving HBM
    traffic per core to 8 MiB in + 8 MiB out -> ~47us DMA roofline/core.
  - Flatten (B,C) -> 32 slabs of (64,128,128); core i takes 4 = 2 PAIRS.
    A pair of slabs is processed together with SBUF partitions = (d, tp)
    (tp = slab parity).  Compared to (d, h-half) partitions this keeps
    per-partition DRAM runs at 16KB on input and 8KB on output (the h
    dimension stays whole in the free dim), so every DMA descriptor is
    large.
  - TensorEngine: one constant 128x128 matrix does the D-axis butterfly on
    the partition axis: M[(2dp+p)*2+tp, a*64+dp*2+tp] = f_a[p]/2 (full
    1/(2*sqrt2) scale folded in).  The matmul's moving-operand access
    pattern feeds columns in (hb, q, r, w2) order, so PSUM lands with the
    W-axis parity r DEINTERLEAVED -- both later butterflies then read/write
    unit-stride bf16 and hit the DVE 2x packed mode.
  - ScalarE drains PSUM f32 -> SBUF bf16 (cast on copy).
  - DVE: H butterfly (pair q blocks, stride-1) then W butterfly (pair r
    blocks, stride-1), all bf16 2x mode.
  - Input DMAs on SP (HWDGE), output DMAs on GPSIMD (SWDGE): each ring has
    one producer so issues never queue behind each other.
"""

import numpy as np
import ml_dtypes

_B, _C, _D, _H, _W = 2, 16, 64, 128, 128
_NCORES = 8
_SLABS = _B * _C  # 32
_SLABS_PER_CORE = _SLABS // _NCORES  # 4
_PAIRS = _SLABS_PER_CORE // 2  # 2

_BF16 = ml_dtypes.bfloat16


def _haar_filters_np():
    # Bit-identical construction to the reference filter bank.
    s = 1.0 / np.sqrt(2.0)
    L = np.array([s, s], dtype=np.float32)
    H = np.array([s, -s], dtype=np.float32)
    bands = [(a, b, c) for a in "LH" for b in "LH" for c in "LH"]
    filt = np.stack(
        [
            (L if a == "L" else H)[:, None, None]
            * (L if b == "L" else H)[None, :, None]
            * (L if c == "L" else H)[None, None, :]
            for (a, b, c) in bands
        ],
        axis=0,
    )  # (8, 2, 2, 2) float32
    return filt


def _haar_matrix():
    """(128,128) for the D-axis butterfly on the partition axis.

    Input partition  = d*2 + tp          (tp = slab parity, d = depth 0..63)
    Output partition = tp*64 + dp*2 + a  (a = D band, dp = 0..31)
    (tp-major over dp so the output DMA's DRAM-side (tp, dp) dims merge into
    one 64-long OUTER dim -- 3-dim AP and a full 16-engine SDMA spray; an
    outer dim of 2 would put the whole transfer on 2 engines.)
    Entry = f_a[p] * s * s  (d = 2dp+p): the full 1/(2*sqrt2) magnitude is
    folded here so the H/W butterflies on DVE are pure +/- adds."""
    filt = _haar_filters_np()
    M = np.zeros((128, 128), dtype=np.float32)
    for tp in range(2):
        for a in range(2):
            for dp in range(32):
                for p in range(2):
                    M[(2 * dp + p) * 2 + tp, tp * 64 + dp * 2 + a] = filt[
                        a * 4, p, 0, 0
                    ]
    return M


def _build_bass():
    import concourse.mybir as mybir
    import concourse.tile as tile
    from concourse import bacc

    f32 = mybir.dt.float32
    bf16 = mybir.dt.bfloat16
    nc = bacc.Bacc("TRN2", target_bir_lowering=False, debug=False)

    x = nc.dram_tensor("x", [_SLABS_PER_CORE, _D, _H, _W], bf16, kind="ExternalInput")
    hm = nc.dram_tensor("hm", [128, 128], bf16, kind="ExternalInput")
    y = nc.dram_tensor(
        "y", [8, _SLABS_PER_CORE, _D // 2, _H // 2, _W // 2], bf16,
        kind="ExternalOutput",
    )

    # x[t=2pr+tp, d, h, w] with h = hh*64 + hb*2 + q, w = w2*2 + r.
    # Half-pair tile (pr, hh): partitions (d, tp), free (hb, q, w) -- each
    # partition's free dim walks a CONTIGUOUS 16KB HBM region; split into two
    # 1MB DMAs (hb halves, 8KB/partition each) so matmuls start early.
    xr = x[:, :, :, :].rearrange(
        "(pr tp) d (hh hb q) w -> pr d tp hh hb q w", tp=2, hh=2, hb=32, q=2
    )
    # y[s=(a,b,g), t=2pr+tp, dp, hp=(hh,hb), wp]; partition order (tp, dp, a);
    # one DMA per (b, g, pr, hh): DRAM dims ((tp dp):64, a:2, (hb wp):2048) --
    # 4KB contiguous per partition, 64-long outer dim for the engine spray.
    yv = y[:, :, :, :, :].rearrange(
        "(a b g) (pr tp) dp (hh hb) wp -> b g pr hh tp dp a hb wp",
        a=2, b=2, tp=2, hh=2,
    )

    with tile.TileContext(nc) as tc:
        with (
            tc.tile_pool(name="const", bufs=1) as cpool,
            tc.tile_pool(name="xin", bufs=3) as xpool,
            tc.tile_pool(name="stage", bufs=2) as spool,
            tc.tile_pool(name="uband", bufs=1) as upool,
            tc.tile_pool(name="outs", bufs=2) as opool,
            tc.tile_pool(name="psum", bufs=2, space="PSUM") as ppool,
        ):
            hmt = cpool.tile([128, 128], bf16, tag="hm")
            nc.sync.dma_start(out=hmt[:, :], in_=hm[:, :])

            def load_half_pair(pr, hh):
                xh = xpool.tile([128, 8192], bf16, tag="xt", name=f"xt_{pr}_{hh}")
                for c in range(2):
                    nc.sync.dma_start(
                        out=xh[:, c * 4096 : (c + 1) * 4096],
                        in_=xr[pr, :, :, hh, c * 16 : (c + 1) * 16],
                    )
                return xh

            HP = [(pr, hh) for pr in range(_PAIRS) for hh in range(2)]
            tiles = {HP[0]: load_half_pair(*HP[0]), HP[1]: load_half_pair(*HP[1])}

            for k, (pr, hh) in enumerate(HP):
                if k + 2 < len(HP):
                    tiles[HP[k + 2]] = load_half_pair(*HP[k + 2])
                xt = tiles.pop((pr, hh))

                # Moving-operand AP in deinterleaved (hb, q, r, w2) order:
                # PSUM layout = (hb, q, r, w2).
                xv = xt[:, :].rearrange(
                    "m (hb q w2 r) -> m hb q r w2", hb=32, q=2, r=2
                )
                st = spool.tile([128, 8192], bf16, tag="st", name=f"st_{pr}_{hh}")
                for c in range(4):  # 2048-wide chunks: hb in [8c, 8c+8)
                    pt = ppool.tile([128, 2048], f32, tag="pt")
                    for j in range(4):  # N=512 matmuls (one PSUM bank each)
                        nc.tensor.matmul(
                            pt[:, j * 512 : (j + 1) * 512],
                            hmt[:, :],
                            xv[:, c * 8 + 2 * j : c * 8 + 2 * j + 2],
                            start=True,
                            stop=True,
                        )
                    # PSUM f32 -> SBUF bf16 (ScalarE, otherwise idle).
                    nc.scalar.copy(st[:, c * 2048 : (c + 1) * 2048], pt[:, :])

                # H butterfly on DVE: pair q=0/q=1 blocks of (r, w2)=128.
                sr = st[:, :].rearrange("m (hb q rw) -> m hb q rw", hb=32, q=2)
                ev, od = sr[:, :, 0, :], sr[:, :, 1, :]
                u = [
                    upool.tile([128, 4096], bf16, tag=f"u{b}", name=f"u{b}_{pr}_{hh}")
                    for b in range(2)
                ]
                u0s = u[0][:, :].rearrange("m (hb rw) -> m hb rw", hb=32)
                u1s = u[1][:, :].rearrange("m (hb rw) -> m hb rw", hb=32)
                nc.vector.tensor_add(u0s, ev, od)
                nc.vector.tensor_sub(u1s, ev, od)

                # W butterfly on DVE (pair r=0/r=1 blocks of w2=64); each
                # band's two subband tiles go straight out once written.
                for b in range(2):
                    ur = u[b][:, :].rearrange("m (hb r w2) -> m hb r w2", hb=32, r=2)
                    uev, uod = ur[:, :, 0, :], ur[:, :, 1, :]
                    o = [
                        opool.tile(
                            [128, 2048], bf16, tag=f"o{b}{g}",
                            name=f"o{b}{g}_{pr}_{hh}",
                        )
                        for g in range(2)
                    ]
                    o0 = o[0][:, :].rearrange("m (hb w2) -> m hb w2", hb=32)
                    o1 = o[1][:, :].rearrange("m (hb w2) -> m hb w2", hb=32)
                    nc.vector.tensor_add(o0, uev, uod)
                    nc.vector.tensor_sub(o1, uev, uod)
                    for g in range(2):
                        # SWDGE (GPSIMD) ring is otherwise idle; SP joins in
                        # once its input issues are done.
                        eng = nc.sync if (pr == _PAIRS - 1 and g == 1) else nc.gpsimd
                        eng.dma_start(out=yv[b, g, pr, hh], in_=o[g][:, :])
    nc.compile()
    return nc


_NC_CACHE = None


def _get_nc():
    global _NC_CACHE
    if _NC_CACHE is None:
        _NC_CACHE = _build_bass()
    return _NC_CACHE


def _run(x, trace=False, **spmd_kwargs):
    from concourse.bass_utils import run_bass_kernel_spmd

    x = np.ascontiguousarray(x, dtype=np.float32)
    xf = x.reshape(_SLABS, _D, _H, _W).astype(_BF16)
    M = _haar_matrix().astype(_BF16)
    in_maps = [
        {
            "x": np.ascontiguousarray(
                xf[i * _SLABS_PER_CORE : (i + 1) * _SLABS_PER_CORE]
            ),
            "hm": M,
        }
        for i in range(_NCORES)
    ]
    res = run_bass_kernel_spmd(
        _get_nc(), in_maps, core_ids=list(range(_NCORES)), trace=trace, **spmd_kwargs
    )
    outs = [r["y"] for r in res.results]  # each (8, 4, 32, 64, 64) bf16
    full = np.concatenate(outs, axis=1).astype(np.float32)  # (8, 32, 32, 64, 64)
    full = full.reshape(8, _B, _C, _D // 2, _H // 2, _W // 2)
    return full, res


def kernel(**inputs):
    full, _ = _run(inputs["x"])
    return tuple(full[i] for i in range(8))


# revision 14
# speedup vs baseline: 1.8656x; 1.0174x over previous
"""3D Haar DWT (2x2x2 blocks, 8 subbands) on 8 Trainium2 NeuronCores.

Input  x: (2, 16, 64, 128, 128) f32.
Output: tuple of 8 subbands, each (2, 16, 32, 64, 64) f32, subband order
LLL,LLH,LHL,LHH,HLL,HLH,HHL,HHH (filters applied to (D,H,W) resp.).

Strategy (pure data parallel, zero cross-core communication), v2 = bf16:
  - The 2e-2 rel-err budget admits bf16 I/O (measured ~2e-3), halving HBM
    traffic per core to 8 MiB in + 8 MiB out -> ~47us DMA roofline/core.
  - Flatten (B,C) -> 32 slabs of (64,128,128); core i takes 4 = 2 PAIRS.
    A pair of slabs is processed together with SBUF partitions = (d, tp)
    (tp = slab parity).  Compared to (d, h-half) partitions this keeps
    per-partition DRAM runs at 16KB on input and 8KB on output (the h
    dimension stays whole in the free dim), so every DMA descriptor is
    large.
  - TensorEngine does the D-axis AND H-axis butterflies: the constant
    128x128 matrix M (and its negation) does the D butterfly on the
    partition axis, and PSUM accumulation over the two H-parity column
    sets does the H butterfly:
      u0 = M @ x[q=0] + M @ x[q=0],  u1 = M @ x[q=0] + (-M) @ x[q=1].
    The moving-operand access patterns feed columns in (hb, r, w2) order,
    so PSUM lands with the W-axis parity r DEINTERLEAVED.
  - ScalarE drains PSUM f32 -> SBUF bf16 (cast on copy) -- the only engine
    that can, at 1x (PSUM has one read port); it is the pipeline pacer.
  - DVE only does the W butterfly (pair r blocks, stride-1, bf16 2x mode).
  - A garbage-operand matmul warmup burst holds the PE busy ~4us up front
    so the HAM clock gate is at 2.4 GHz when real matmuls arrive.
  - Input DMAs on SP (HWDGE), output DMAs on GPSIMD (SWDGE): each ring has
    one producer so issues never queue behind each other.
"""

import numpy as np
import ml_dtypes

_B, _C, _D, _H, _W = 2, 16, 64, 128, 128
_NCORES = 8
_SLABS = _B * _C  # 32
_SLABS_PER_CORE = _SLABS // _NCORES  # 4
_PAIRS = _SLABS_PER_CORE // 2  # 2

_BF16 = ml_dtypes.bfloat16


def _haar_filters_np():
    # Bit-identical construction to the reference filter bank.
    s = 1.0 / np.sqrt(2.0)
    L = np.array([s, s], dtype=np.float32)
    H = np.array([s, -s], dtype=np.float32)
    bands = [(a, b, c) for a in "LH" for b in "LH" for c in "LH"]
    filt = np.stack(
        [
            (L if a == "L" else H)[:, None, None]
            * (L if b == "L" else H)[None, :, None]
            * (L if c == "L" else H)[None, None, :]
            for (a, b, c) in bands
        ],
        axis=0,
    )  # (8, 2, 2, 2) float32
    return filt


def _haar_matrix():
    """(128,128) for the D-axis butterfly on the partition axis.

    Input partition  = d*2 + tp          (tp = slab parity, d = depth 0..63)
    Output partition = tp*64 + dp*2 + a  (a = D band, dp = 0..31)
    (tp-major over dp so the output DMA's DRAM-side (tp, dp) dims merge into
    one 64-long OUTER dim -- 3-dim AP and a full 16-engine SDMA spray; an
    outer dim of 2 would put the whole transfer on 2 engines.)
    Entry = f_a[p] * s * s  (d = 2dp+p): the full 1/(2*sqrt2) magnitude is
    folded here so the H/W butterflies on DVE are pure +/- adds."""
    filt = _haar_filters_np()
    M = np.zeros((128, 128), dtype=np.float32)
    for tp in range(2):
        for a in range(2):
            for dp in range(32):
                for p in range(2):
                    M[(2 * dp + p) * 2 + tp, tp * 64 + dp * 2 + a] = filt[
                        a * 4, p, 0, 0
                    ]
    return M


def _build_bass():
    import concourse.mybir as mybir
    import concourse.tile as tile
    from concourse import bacc

    f32 = mybir.dt.float32
    bf16 = mybir.dt.bfloat16
    nc = bacc.Bacc("TRN2", target_bir_lowering=False, debug=False)

    x = nc.dram_tensor("x", [_SLABS_PER_CORE, _D, _H, _W], bf16, kind="ExternalInput")
    hm = nc.dram_tensor("hm", [128, 256], bf16, kind="ExternalInput")  # [M | -M]
    y = nc.dram_tensor(
        "y", [8, _SLABS_PER_CORE, _D // 2, _H // 2, _W // 2], bf16,
        kind="ExternalOutput",
    )

    # x[t=2pr+tp, d, h, w] with h = hh*64 + hb*2 + q, w = w2*2 + r.
    # Half-pair tile (pr, hh): partitions (d, tp), free (hb, q, w) -- each
    # partition's free dim walks a CONTIGUOUS 16KB HBM region; split into two
    # 1MB DMAs (hb halves, 8KB/partition each) so matmuls start early.
    xr = x[:, :, :, :].rearrange(
        "(pr tp) d (hh hb q) w -> pr d tp hh hb q w", tp=2, hh=2, hb=32, q=2
    )
    # y[s=(a,b,g), t=2pr+tp, dp, hp=(hh,hb), wp]; partition order (tp, dp, a);
    # one DMA per (b, g, pr, hh): DRAM dims ((tp dp):64, a:2, (hb wp):2048) --
    # 4KB contiguous per partition, 64-long outer dim for the engine spray.
    yv = y[:, :, :, :, :].rearrange(
        "(a b g) (pr tp) dp (hh hb) wp -> b g pr hh tp dp a hb wp",
        a=2, b=2, tp=2, hh=2,
    )

    with tile.TileContext(nc) as tc:
        with (
            tc.tile_pool(name="const", bufs=1) as cpool,
            tc.tile_pool(name="xin", bufs=3) as xpool,
            tc.tile_pool(name="uband", bufs=2) as upool,
            tc.tile_pool(name="outs", bufs=2) as opool,
            tc.tile_pool(name="psum", bufs=2, space="PSUM") as ppool,
        ):
            hmt = cpool.tile([128, 256], bf16, tag="hm")
            nc.sync.dma_start(out=hmt[:, :], in_=hm[:, :])
            hmp, hmn = hmt[:, 0:128], hmt[:, 128:256]

            # PE warmup: ~4.5us of garbage matmuls flips the HAM clock gate
            # to 8/8 (2.4 GHz) before the first real matmul; operands are the
            # already-loaded filter tile (values irrelevant, never drained).
            wp = ppool.tile([128, 1024], f32, tag="p0", name="warm")
            for i in range(14):
                nc.tensor.matmul(
                    wp[:, 0:256], hmt[:, 0:128], hmt[:, 0:256],
                    start=True, stop=True,
                )

            def load_half_pair(pr, hh):
                xh = xpool.tile([128, 8192], bf16, tag="xt", name=f"xt_{pr}_{hh}")
                for c in range(2):
                    nc.sync.dma_start(
                        out=xh[:, c * 4096 : (c + 1) * 4096],
                        in_=xr[pr, :, :, hh, c * 16 : (c + 1) * 16],
                    )
                return xh

            HP = [(pr, hh) for pr in range(_PAIRS) for hh in range(2)]
            tiles = {HP[0]: load_half_pair(*HP[0]), HP[1]: load_half_pair(*HP[1])}

            for k, (pr, hh) in enumerate(HP):
                if k + 2 < len(HP):
                    tiles[HP[k + 2]] = load_half_pair(*HP[k + 2])
                xt = tiles.pop((pr, hh))

                # Moving-operand APs, H parity q split out, remaining column
                # order (hb, r, w2): PSUM lands W-deinterleaved.
                xv = xt[:, :].rearrange(
                    "m (hb q w2 r) -> m q hb r w2", hb=32, q=2, r=2
                )
                # u tiles: post D+H data, free (hb, r, w2).
                u = [
                    upool.tile([128, 4096], bf16, tag=f"u{b}", name=f"u{b}_{pr}_{hh}")
                    for b in range(2)
                ]
                o = [
                    [
                        opool.tile(
                            [128, 2048], bf16, tag=f"o{b}{g}",
                            name=f"o{b}{g}_{pr}_{hh}",
                        )
                        for g in range(2)
                    ]
                    for b in range(2)
                ]

                def w_stage(s):
                    # W butterfly on DVE for semi s (hb in [16s, 16s+16)):
                    # pair r=0/r=1 blocks of w2=64, all stride-1 bf16 (2x).
                    for b in range(2):
                        ur = u[b][:, s * 2048 : (s + 1) * 2048].rearrange(
                            "m (hb r w2) -> m hb r w2", hb=16, r=2
                        )
                        uev, uod = ur[:, :, 0, :], ur[:, :, 1, :]
                        o0 = o[b][0][:, s * 1024 : (s + 1) * 1024].rearrange(
                            "m (hb w2) -> m hb w2", hb=16
                        )
                        o1 = o[b][1][:, s * 1024 : (s + 1) * 1024].rearrange(
                            "m (hb w2) -> m hb w2", hb=16
                        )
                        nc.vector.tensor_add(o0, uev, uod)
                        nc.vector.tensor_sub(o1, uev, uod)
                        if s == 1:
                            for g in range(2):
                                # SWDGE ring is otherwise idle; SP joins in
                                # once its input issues are done.
                                eng = (
                                    nc.sync
                                    if (pr == _PAIRS - 1 and g == 1)
                                    else nc.gpsimd
                                )
                                eng.dma_start(
                                    out=yv[b, g, pr, hh], in_=o[b][g][:, :]
                                )

                for m in range(4):  # mini-quads: hb in [8m, 8m+8)
                    pu = [
                        ppool.tile([128, 1024], f32, tag=f"p{b}", name=f"p{b}_{k}_{m}")
                        for b in range(2)
                    ]
                    # H butterfly via PSUM accumulation: q=0 pass (+M) then
                    # q=1 pass (+M into u0, -M into u1).  LDWEIGHTS-friendly
                    # order: all +M matmuls first.
                    for b in range(2):
                        for c in range(2):  # bank c: hb in [8m+4c, 8m+4c+4)
                            nc.tensor.matmul(
                                pu[b][:, c * 512 : (c + 1) * 512],
                                hmp,
                                xv[:, 0, 4 * (2 * m + c) : 4 * (2 * m + c) + 4],
                                start=True,
                                stop=False,
                            )
                    for b in range(2):
                        for c in range(2):
                            nc.tensor.matmul(
                                pu[b][:, c * 512 : (c + 1) * 512],
                                (hmp, hmn)[b],
                                xv[:, 1, 4 * (2 * m + c) : 4 * (2 * m + c) + 4],
                                start=False,
                                stop=True,
                            )
                    # PSUM f32 -> SBUF bf16 (ScalarE -- the 1x pacer).
                    for b in range(2):
                        nc.scalar.copy(
                            u[b][:, m * 1024 : (m + 1) * 1024], pu[b][:, :]
                        )
                    if m == 1:
                        w_stage(0)
                w_stage(1)
    nc.compile()
    return nc


_NC_CACHE = None


def _get_nc():
    global _NC_CACHE
    if _NC_CACHE is None:
        _NC_CACHE = _build_bass()
    return _NC_CACHE


def _run(x, trace=False, **spmd_kwargs):
    from concourse.bass_utils import run_bass_kernel_spmd

    x = np.ascontiguousarray(x, dtype=np.float32)
    xf = x.reshape(_SLABS, _D, _H, _W).astype(_BF16)
    M = _haar_matrix()
    Mpn = np.ascontiguousarray(np.concatenate([M, -M], axis=1)).astype(_BF16)
    in_maps = [
        {
            "x": np.ascontiguousarray(
                xf[i * _SLABS_PER_CORE : (i + 1) * _SLABS_PER_CORE]
            ),
            "hm": Mpn,
        }
        for i in range(_NCORES)
    ]
    res = run_bass_kernel_spmd(
        _get_nc(), in_maps, core_ids=list(range(_NCORES)), trace=trace, **spmd_kwargs
    )
    outs = [r["y"] for r in res.results]  # each (8, 4, 32, 64, 64) bf16
    full = np.concatenate(outs, axis=1).astype(np.float32)  # (8, 32, 32, 64, 64)
    full = full.reshape(8, _B, _C, _D // 2, _H // 2, _W // 2)
    return full, res


def kernel(**inputs):
    full, _ = _run(inputs["x"])
    return tuple(full[i] for i in range(8))


# revision 17
# speedup vs baseline: 1.9997x; 1.0719x over previous
"""3D Haar DWT (2x2x2 blocks, 8 subbands) on 8 Trainium2 NeuronCores.

Input  x: (2, 16, 64, 128, 128) f32.
Output: tuple of 8 subbands, each (2, 16, 32, 64, 64) f32, subband order
LLL,LLH,LHL,LHH,HLL,HLH,HHL,HHH (filters applied to (D,H,W) resp.).

Strategy (pure data parallel, zero cross-core communication), v2 = bf16:
  - The 2e-2 rel-err budget admits bf16 I/O (measured ~2e-3), halving HBM
    traffic per core to 8 MiB in + 8 MiB out -> ~47us DMA roofline/core.
  - Flatten (B,C) -> 32 slabs of (64,128,128); core i takes 4 = 2 PAIRS.
    A pair of slabs is processed together with SBUF partitions = (d, tp)
    (tp = slab parity).  Compared to (d, h-half) partitions this keeps
    per-partition DRAM runs at 16KB on input and 8KB on output (the h
    dimension stays whole in the free dim), so every DMA descriptor is
    large.
  - TensorEngine does the D-axis AND H-axis butterflies: the constant
    128x128 matrix M (and its negation) does the D butterfly on the
    partition axis, and PSUM accumulation over the two H-parity column
    sets does the H butterfly:
      u0 = M @ x[q=0] + M @ x[q=0],  u1 = M @ x[q=0] + (-M) @ x[q=1].
    The moving-operand access patterns feed columns in (hb, r, w2) order,
    so PSUM lands with the W-axis parity r DEINTERLEAVED.
  - ScalarE drains PSUM f32 -> SBUF bf16 (cast on copy) -- the only engine
    that can, at 1x (PSUM has one read port); it is the pipeline pacer.
  - DVE only does the W butterfly (pair r blocks, stride-1, bf16 2x mode).
  - A garbage-operand matmul warmup burst holds the PE busy ~4us up front
    so the HAM clock gate is at 2.4 GHz when real matmuls arrive.
  - Input DMAs on SP (HWDGE), output DMAs on GPSIMD (SWDGE): each ring has
    one producer so issues never queue behind each other.
"""

import numpy as np
import ml_dtypes

_B, _C, _D, _H, _W = 2, 16, 64, 128, 128
_NCORES = 8
_SLABS = _B * _C  # 32
_SLABS_PER_CORE = _SLABS // _NCORES  # 4
_PAIRS = _SLABS_PER_CORE // 2  # 2

_BF16 = ml_dtypes.bfloat16


def _haar_filters_np():
    # Bit-identical construction to the reference filter bank.
    s = 1.0 / np.sqrt(2.0)
    L = np.array([s, s], dtype=np.float32)
    H = np.array([s, -s], dtype=np.float32)
    bands = [(a, b, c) for a in "LH" for b in "LH" for c in "LH"]
    filt = np.stack(
        [
            (L if a == "L" else H)[:, None, None]
            * (L if b == "L" else H)[None, :, None]
            * (L if c == "L" else H)[None, None, :]
            for (a, b, c) in bands
        ],
        axis=0,
    )  # (8, 2, 2, 2) float32
    return filt


def _haar_matrix():
    """(128,128) for the D-axis butterfly on the partition axis.

    Input partition  = d*2 + tp          (tp = slab parity, d = depth 0..63)
    Output partition = tp*64 + dp*2 + a  (a = D band, dp = 0..31)
    (tp-major over dp so the output DMA's DRAM-side (tp, dp) dims merge into
    one 64-long OUTER dim -- 3-dim AP and a full 16-engine SDMA spray; an
    outer dim of 2 would put the whole transfer on 2 engines.)
    Entry = f_a[p] * s * s  (d = 2dp+p): the full 1/(2*sqrt2) magnitude is
    folded here so the H/W butterflies on DVE are pure +/- adds."""
    filt = _haar_filters_np()
    M = np.zeros((128, 128), dtype=np.float32)
    for tp in range(2):
        for a in range(2):
            for dp in range(32):
                for p in range(2):
                    M[(2 * dp + p) * 2 + tp, tp * 64 + dp * 2 + a] = filt[
                        a * 4, p, 0, 0
                    ]
    return M


def _build_bass():
    import concourse.mybir as mybir
    import concourse.tile as tile
    from concourse import bacc

    f32 = mybir.dt.float32
    bf16 = mybir.dt.bfloat16
    nc = bacc.Bacc("TRN2", target_bir_lowering=False, debug=False)

    x = nc.dram_tensor("x", [_SLABS_PER_CORE, _D, _H, _W], bf16, kind="ExternalInput")
    hm = nc.dram_tensor("hm", [128, 256], bf16, kind="ExternalInput")  # [M | -M]
    y = nc.dram_tensor(
        "y", [8, _SLABS_PER_CORE, _D // 2, _H // 2, _W // 2], bf16,
        kind="ExternalOutput",
    )

    # x[t=2pr+tp, d, h, w] with h = hh*64 + hb*2 + q, w = w2*2 + r.
    # Half-pair tile (pr, hh): partitions (d, tp), free (hb, q, w) -- each
    # partition's free dim walks a CONTIGUOUS 16KB HBM region; split into two
    # 1MB DMAs (hb halves, 8KB/partition each) so matmuls start early.
    xr = x[:, :, :, :].rearrange(
        "(pr tp) d (hh hb q) w -> pr d tp hh hb q w", tp=2, hh=2, hb=32, q=2
    )
    # y[s=(a,b,g), t=2pr+tp, dp, hp=(hh,hb), wp]; partition order (tp, dp, a);
    # one DMA per (b, g, pr, hh): DRAM dims ((tp dp):64, a:2, (hb wp):2048) --
    # 4KB contiguous per partition, 64-long outer dim for the engine spray.
    yv = y[:, :, :, :, :].rearrange(
        "(a b g) (pr tp) dp (hh hb) wp -> b g pr hh tp dp a hb wp",
        a=2, b=2, tp=2, hh=2,
    )

    with tile.TileContext(nc) as tc:
        with (
            tc.tile_pool(name="const", bufs=1) as cpool,
            tc.tile_pool(name="xin", bufs=3) as xpool,
            tc.tile_pool(name="uband", bufs=2) as upool,
            tc.tile_pool(name="outs", bufs=2) as opool,
            tc.tile_pool(name="psum", bufs=2, space="PSUM") as ppool,
        ):
            hmt = cpool.tile([128, 256], bf16, tag="hm")
            nc.sync.dma_start(out=hmt[:, :], in_=hm[:, :])
            hmp, hmn = hmt[:, 0:128], hmt[:, 128:256]

            # PE warmup: ~4.5us of garbage matmuls flips the HAM clock gate
            # to 8/8 (2.4 GHz) before the first real matmul; operands are the
            # already-loaded filter tile (values irrelevant, never drained).
            wp = ppool.tile([128, 2048], f32, tag="pq", name="warm")
            for i in range(14):
                nc.tensor.matmul(
                    wp[:, 0:256], hmt[:, 0:128], hmt[:, 0:256],
                    start=True, stop=True,
                )

            def load_half_pair(pr, hh):
                xh = xpool.tile([128, 8192], bf16, tag="xt", name=f"xt_{pr}_{hh}")
                for c in range(2):
                    nc.sync.dma_start(
                        out=xh[:, c * 4096 : (c + 1) * 4096],
                        in_=xr[pr, :, :, hh, c * 16 : (c + 1) * 16],
                    )
                return xh

            HP = [(pr, hh) for pr in range(_PAIRS) for hh in range(2)]
            tiles = {HP[0]: load_half_pair(*HP[0]), HP[1]: load_half_pair(*HP[1])}

            for k, (pr, hh) in enumerate(HP):
                if k + 2 < len(HP):
                    tiles[HP[k + 2]] = load_half_pair(*HP[k + 2])
                xt = tiles.pop((pr, hh))

                # Moving-operand APs, H parity q split out, remaining column
                # order (hb, r, w2): PSUM lands W-deinterleaved.
                xv = xt[:, :].rearrange(
                    "m (hb q w2 r) -> m q hb r w2", hb=32, q=2, r=2
                )
                # u tile: post D+H data, free (mq: mini-quad 4, b: H band 2,
                # hb: 8, r: 2, w2: 64) -- one ScalarE drain per mini-quad.
                ub = upool.tile([128, 8192], bf16, tag="ub", name=f"ub_{pr}_{hh}")
                o = [
                    [
                        opool.tile(
                            [128, 2048], bf16, tag=f"o{b}{g}",
                            name=f"o{b}{g}_{pr}_{hh}",
                        )
                        for g in range(2)
                    ]
                    for b in range(2)
                ]

                uv = ub[:, :].rearrange(
                    "m (mq b hb r w2) -> m mq b hb r w2", mq=4, b=2, hb=8, r=2
                )

                def w_stage(s):
                    # W butterfly on DVE for semi s (mini-quads 2s, 2s+1):
                    # pair r=0/r=1 blocks of w2=64, all stride-1 bf16 (2x).
                    for b in range(2):
                        uev = uv[:, 2 * s : 2 * s + 2, b, :, 0, :]
                        uod = uv[:, 2 * s : 2 * s + 2, b, :, 1, :]
                        o0 = o[b][0][:, s * 1024 : (s + 1) * 1024].rearrange(
                            "m (mq hb w2) -> m mq hb w2", mq=2, hb=8
                        )
                        o1 = o[b][1][:, s * 1024 : (s + 1) * 1024].rearrange(
                            "m (mq hb w2) -> m mq hb w2", mq=2, hb=8
                        )
                        nc.vector.tensor_add(o0, uev, uod)
                        nc.vector.tensor_sub(o1, uev, uod)
                        if s == 1:
                            for g in range(2):
                                # SWDGE ring is otherwise idle; SP joins in
                                # once its input issues are done.
                                eng = (
                                    nc.sync
                                    if (pr == _PAIRS - 1 and g == 1)
                                    else nc.gpsimd
                                )
                                eng.dma_start(
                                    out=yv[b, g, pr, hh], in_=o[b][g][:, :]
                                )

                for m in range(4):  # mini-quads: hb in [8m, 8m+8)
                    pq = ppool.tile([128, 2048], f32, tag="pq", name=f"pq_{k}_{m}")
                    # H butterfly via PSUM accumulation: q=0 pass (+M) then
                    # q=1 pass (+M into u0 half, -M into u1 half).
                    # LDWEIGHTS-friendly order: all +M matmuls first.
                    for b in range(2):
                        for c in range(2):  # bank: hb in [8m+4c, 8m+4c+4)
                            nc.tensor.matmul(
                                pq[:, b * 1024 + c * 512 : b * 1024 + c * 512 + 512],
                                hmp,
                                xv[:, 0, 4 * (2 * m + c) : 4 * (2 * m + c) + 4],
                                start=True,
                                stop=False,
                            )
                    for b in range(2):
                        for c in range(2):
                            nc.tensor.matmul(
                                pq[:, b * 1024 + c * 512 : b * 1024 + c * 512 + 512],
                                (hmp, hmn)[b],
                                xv[:, 1, 4 * (2 * m + c) : 4 * (2 * m + c) + 4],
                                start=False,
                                stop=True,
                            )
                    # PSUM f32 -> SBUF bf16 (ScalarE -- the 1x pacer).
                    nc.scalar.copy(ub[:, m * 2048 : (m + 1) * 2048], pq[:, :])
                    if m == 1:
                        w_stage(0)
                w_stage(1)
    nc.compile()
    return nc


_NC_CACHE = None


def _get_nc():
    global _NC_CACHE
    if _NC_CACHE is None:
        _NC_CACHE = _build_bass()
    return _NC_CACHE


def _run(x, trace=False, **spmd_kwargs):
    from concourse.bass_utils import run_bass_kernel_spmd

    x = np.ascontiguousarray(x, dtype=np.float32)
    xf = x.reshape(_SLABS, _D, _H, _W).astype(_BF16)
    M = _haar_matrix()
    Mpn = np.ascontiguousarray(np.concatenate([M, -M], axis=1)).astype(_BF16)
    in_maps = [
        {
            "x": np.ascontiguousarray(
                xf[i * _SLABS_PER_CORE : (i + 1) * _SLABS_PER_CORE]
            ),
            "hm": Mpn,
        }
        for i in range(_NCORES)
    ]
    res = run_bass_kernel_spmd(
        _get_nc(), in_maps, core_ids=list(range(_NCORES)), trace=trace, **spmd_kwargs
    )
    outs = [r["y"] for r in res.results]  # each (8, 4, 32, 64, 64) bf16
    full = np.concatenate(outs, axis=1).astype(np.float32)  # (8, 32, 32, 64, 64)
    full = full.reshape(8, _B, _C, _D // 2, _H // 2, _W // 2)
    return full, res


def kernel(**inputs):
    full, _ = _run(inputs["x"])
    return tuple(full[i] for i in range(8))


# revision 23
# speedup vs baseline: 2.1471x; 1.0737x over previous
"""3D Haar DWT (2x2x2 blocks, 8 subbands) on 8 Trainium2 NeuronCores.

Input  x: (2, 16, 64, 128, 128) f32.
Output: tuple of 8 subbands, each (2, 16, 32, 64, 64) f32, subband order
LLL,LLH,LHL,LHH,HLL,HLH,HHL,HHH (filters applied to (D,H,W) resp.).

Strategy (pure data parallel, zero cross-core communication), v2 = bf16:
  - The 2e-2 rel-err budget admits bf16 I/O (measured ~2e-3), halving HBM
    traffic per core to 8 MiB in + 8 MiB out -> ~47us DMA roofline/core.
  - Flatten (B,C) -> 32 slabs of (64,128,128); core i takes 4 = 2 PAIRS.
    A pair of slabs is processed together with SBUF partitions = (d, tp)
    (tp = slab parity).  Compared to (d, h-half) partitions this keeps
    per-partition DRAM runs at 16KB on input and 8KB on output (the h
    dimension stays whole in the free dim), so every DMA descriptor is
    large.
  - TensorEngine does the D-axis AND H-axis butterflies: the constant
    128x128 matrix M (and its negation) does the D butterfly on the
    partition axis, and PSUM accumulation over the two H-parity column
    sets does the H butterfly:
      u0 = M @ x[q=0] + M @ x[q=0],  u1 = M @ x[q=0] + (-M) @ x[q=1].
    The moving-operand access patterns feed columns in (hb, r, w2) order,
    so PSUM lands with the W-axis parity r DEINTERLEAVED.
  - ScalarE drains PSUM f32 -> SBUF bf16 (cast on copy) -- the only engine
    that can, at 1x (PSUM has one read port); it is the pipeline pacer.
  - DVE only does the W butterfly (pair r blocks, stride-1, bf16 2x mode).
  - A garbage-operand matmul warmup burst holds the PE busy ~4us up front
    so the HAM clock gate is at 2.4 GHz when real matmuls arrive.
  - Input DMAs on SP (HWDGE), output DMAs on GPSIMD (SWDGE): each ring has
    one producer so issues never queue behind each other.
"""

import numpy as np
import ml_dtypes

_B, _C, _D, _H, _W = 2, 16, 64, 128, 128
_NCORES = 8
_SLABS = _B * _C  # 32
_SLABS_PER_CORE = _SLABS // _NCORES  # 4
_PAIRS = _SLABS_PER_CORE // 2  # 2

_BF16 = ml_dtypes.bfloat16


def _haar_filters_np():
    # Bit-identical construction to the reference filter bank.
    s = 1.0 / np.sqrt(2.0)
    L = np.array([s, s], dtype=np.float32)
    H = np.array([s, -s], dtype=np.float32)
    bands = [(a, b, c) for a in "LH" for b in "LH" for c in "LH"]
    filt = np.stack(
        [
            (L if a == "L" else H)[:, None, None]
            * (L if b == "L" else H)[None, :, None]
            * (L if c == "L" else H)[None, None, :]
            for (a, b, c) in bands
        ],
        axis=0,
    )  # (8, 2, 2, 2) float32
    return filt


def _haar_matrix():
    """(128,128) for the D-axis butterfly on the partition axis.

    Input partition  = d*2 + tp          (tp = slab parity, d = depth 0..63)
    Output partition = tp*64 + dp*2 + a  (a = D band, dp = 0..31)
    (tp-major over dp so the output DMA's DRAM-side (tp, dp) dims merge into
    one 64-long OUTER dim -- 3-dim AP and a full 16-engine SDMA spray; an
    outer dim of 2 would put the whole transfer on 2 engines.)
    Entry = f_a[p] * s * s  (d = 2dp+p): the full 1/(2*sqrt2) magnitude is
    folded here so the H/W butterflies on DVE are pure +/- adds."""
    filt = _haar_filters_np()
    M = np.zeros((128, 128), dtype=np.float32)
    for tp in range(2):
        for a in range(2):
            for dp in range(32):
                for p in range(2):
                    M[(2 * dp + p) * 2 + tp, tp * 64 + dp * 2 + a] = filt[
                        a * 4, p, 0, 0
                    ]
    return M


def _build_bass():
    import concourse.mybir as mybir
    import concourse.tile as tile
    from concourse import bacc

    f32 = mybir.dt.float32
    bf16 = mybir.dt.bfloat16
    nc = bacc.Bacc("TRN2", target_bir_lowering=False, debug=False)

    x = nc.dram_tensor("x", [_SLABS_PER_CORE, _D, _H, _W], bf16, kind="ExternalInput")
    hm = nc.dram_tensor("hm", [128, 256], bf16, kind="ExternalInput")  # [M | -M]
    y = nc.dram_tensor(
        "y", [8, _SLABS_PER_CORE, _D // 2, _H // 2, _W // 2], bf16,
        kind="ExternalOutput",
    )

    # x[t=2pr+tp, d, h, w] with h = hh*64 + hb*2 + q, w = w2*2 + r.
    # Half-pair tile (pr, hh): partitions (d, tp), free (hb, q, w) -- each
    # partition's free dim walks a CONTIGUOUS 16KB HBM region; split into two
    # 1MB DMAs (hb halves, 8KB/partition each) so matmuls start early.
    xr = x[:, :, :, :].rearrange(
        "(pr tp) d (hh hb q) w -> pr d tp hh hb q w", tp=2, hh=2, hb=32, q=2
    )
    # y[s=(a,b,g), t=2pr+tp, dp, hp=(hh,hb), wp]; partition order (tp, dp, a);
    # one DMA per (b, g, pr, hh): DRAM dims ((tp dp):64, a:2, (hb wp):2048) --
    # 4KB contiguous per partition, 64-long outer dim for the engine spray.
    yv = y[:, :, :, :, :].rearrange(
        "(a b g) (pr tp) dp (hh hb) wp -> b g pr hh tp dp a hb wp",
        a=2, b=2, tp=2, hh=2,
    )

    with tile.TileContext(nc) as tc:
        with (
            tc.tile_pool(name="const", bufs=1) as cpool,
            tc.tile_pool(name="xin", bufs=3) as xpool,
            tc.tile_pool(name="uband", bufs=2) as upool,
            tc.tile_pool(name="outs", bufs=3) as opool,
            tc.tile_pool(name="psum", bufs=2, space="PSUM") as ppool,
        ):
            hmt = cpool.tile([128, 256], bf16, tag="hm")
            nc.sync.dma_start(out=hmt[:, :], in_=hm[:, :])
            hmp, hmn = hmt[:, 0:128], hmt[:, 128:256]

            # PE warmup: ~4.5us of garbage matmuls flips the HAM clock gate
            # to 8/8 (2.4 GHz) before the first real matmul; operands are the
            # already-loaded filter tile (values irrelevant, never drained).
            wp = ppool.tile([128, 2048], f32, tag="pq", name="warm")
            for i in range(14):
                nc.tensor.matmul(
                    wp[:, 0:256], hmt[:, 0:128], hmt[:, 0:256],
                    start=True, stop=True,
                )

            def load_half_pair(pr, hh):
                xh = xpool.tile([128, 8192], bf16, tag="xt", name=f"xt_{pr}_{hh}")
                for c in range(2):
                    nc.sync.dma_start(
                        out=xh[:, c * 4096 : (c + 1) * 4096],
                        in_=xr[pr, :, :, hh, c * 16 : (c + 1) * 16],
                    )
                return xh

            HP = [(pr, hh) for pr in range(_PAIRS) for hh in range(2)]
            tiles = {HP[0]: load_half_pair(*HP[0]), HP[1]: load_half_pair(*HP[1])}

            for k, (pr, hh) in enumerate(HP):
                if k + 2 < len(HP):
                    tiles[HP[k + 2]] = load_half_pair(*HP[k + 2])
                xt = tiles.pop((pr, hh))

                # Engine balance: half-pair 0 computes the H butterfly on DVE
                # (which would otherwise idle at the start); the rest fold H
                # into the PE via PSUM accumulation (2x matmul passes).
                h_on_dve = k == 0

                # Moving-operand APs, H parity q split out, remaining column
                # order (hb, r, w2): PSUM lands W-deinterleaved.
                xv = xt[:, :].rearrange(
                    "m (hb q w2 r) -> m q hb r w2", hb=32, q=2, r=2
                )
                # Single-pass variant: columns in (hb, q, r, w2) order.
                xv2 = xt[:, :].rearrange(
                    "m (hb q w2 r) -> m hb q r w2", hb=32, q=2, r=2
                )
                # u tile: post D+H data, free (mq: mini-quad 4, b: H band 2,
                # hb: 8, r: 2, w2: 64) -- one ScalarE drain per mini-quad.
                ub = upool.tile([128, 8192], bf16, tag="ub", name=f"ub_{pr}_{hh}")
                o = [
                    [
                        opool.tile(
                            [128, 2048], bf16, tag=f"o{b}{g}",
                            name=f"o{b}{g}_{pr}_{hh}",
                        )
                        for g in range(2)
                    ]
                    for b in range(2)
                ]

                if h_on_dve:
                    # ub holds P (pre-H) in (mq, hb, q, r, w2) layout; DVE's
                    # H butterfly (pair q blocks) writes ud in the standard
                    # (mq, b, hb, r, w2) layout the W stage expects.
                    ud = upool.tile(
                        [128, 8192], bf16, tag="ud", bufs=1, name=f"ud_{pr}_{hh}"
                    )
                    uv = ud[:, :].rearrange(
                        "m (mq b hb r w2) -> m mq b hb r w2", mq=4, b=2, hb=8, r=2
                    )
                    pv = ub[:, :].rearrange(
                        "m (mq hb q rw) -> m mq hb q rw", mq=4, hb=8, q=2
                    )
                    udh = ud[:, :].rearrange(
                        "m (mq b hb rw) -> m mq b hb rw", mq=4, b=2, hb=8
                    )
                else:
                    uv = ub[:, :].rearrange(
                        "m (mq b hb r w2) -> m mq b hb r w2", mq=4, b=2, hb=8, r=2
                    )

                def h_stage(s):
                    # H butterfly on DVE for semi s (h_on_dve half-pairs):
                    # pair q=0/q=1 blocks of (r, w2)=128, stride-1 bf16 (2x).
                    ev = pv[:, 2 * s : 2 * s + 2, :, 0, :]
                    od = pv[:, 2 * s : 2 * s + 2, :, 1, :]
                    u0 = udh[:, 2 * s : 2 * s + 2, 0, :, :]
                    u1 = udh[:, 2 * s : 2 * s + 2, 1, :, :]
                    nc.vector.tensor_add(u0, ev, od)
                    nc.vector.tensor_sub(u1, ev, od)

                def w_stage(s):
                    # W butterfly on DVE for semi s (mini-quads 2s, 2s+1):
                    # pair r=0/r=1 blocks of w2=64, all stride-1 bf16 (2x).
                    if h_on_dve:
                        h_stage(s)
                    for b in range(2):
                        uev = uv[:, 2 * s : 2 * s + 2, b, :, 0, :]
                        uod = uv[:, 2 * s : 2 * s + 2, b, :, 1, :]
                        o0 = o[b][0][:, s * 1024 : (s + 1) * 1024].rearrange(
                            "m (mq hb w2) -> m mq hb w2", mq=2, hb=8
                        )
                        o1 = o[b][1][:, s * 1024 : (s + 1) * 1024].rearrange(
                            "m (mq hb w2) -> m mq hb w2", mq=2, hb=8
                        )
                        nc.vector.tensor_add(o0, uev, uod)
                        nc.vector.tensor_sub(o1, uev, uod)
                        if s == 1:
                            for g in range(2):
                                # SWDGE ring is otherwise idle; SP joins in
                                # once its input issues are done.
                                eng = (
                                    nc.sync
                                    if (pr == _PAIRS - 1 and g == 1)
                                    else nc.gpsimd
                                )
                                eng.dma_start(
                                    out=yv[b, g, pr, hh], in_=o[b][g][:, :]
                                )

                for m in range(4):  # mini-quads: hb in [8m, 8m+8)
                    pq = ppool.tile([128, 2048], f32, tag="pq", name=f"pq_{k}_{m}")
                    if h_on_dve:
                        # Single pass: P = M @ x, H left to DVE.
                        for j in range(4):
                            nc.tensor.matmul(
                                pq[:, j * 512 : (j + 1) * 512],
                                hmp,
                                xv2[:, 8 * m + 2 * j : 8 * m + 2 * j + 2],
                                start=True,
                                stop=True,
                            )
                    else:
                        # H butterfly via PSUM accumulation: q=0 pass (+M)
                        # then q=1 pass (+M into u0 half, -M into u1 half).
                        # LDWEIGHTS-friendly order: all +M matmuls first.
                        for b in range(2):
                            for c in range(2):  # bank: hb in [8m+4c, 8m+4c+4)
                                nc.tensor.matmul(
                                    pq[
                                        :,
                                        b * 1024 + c * 512 : b * 1024 + c * 512 + 512,
                                    ],
                                    hmp,
                                    xv[:, 0, 4 * (2 * m + c) : 4 * (2 * m + c) + 4],
                                    start=True,
                                    stop=False,
                                )
                        for b in range(2):
                            for c in range(2):
                                nc.tensor.matmul(
                                    pq[
                                        :,
                                        b * 1024 + c * 512 : b * 1024 + c * 512 + 512,
                                    ],
                                    (hmp, hmn)[b],
                                    xv[:, 1, 4 * (2 * m + c) : 4 * (2 * m + c) + 4],
                                    start=False,
                                    stop=True,
                                )
                    # PSUM f32 -> SBUF bf16 (ScalarE -- the 1x pacer).
                    nc.scalar.copy(ub[:, m * 2048 : (m + 1) * 2048], pq[:, :])
                    if m == 1:
                        w_stage(0)
                w_stage(1)
    nc.compile()
    return nc


_NC_CACHE = None


def _get_nc():
    global _NC_CACHE
    if _NC_CACHE is None:
        _NC_CACHE = _build_bass()
    return _NC_CACHE


def _run(x, trace=False, **spmd_kwargs):
    from concourse.bass_utils import run_bass_kernel_spmd

    x = np.ascontiguousarray(x, dtype=np.float32)
    xf = x.reshape(_SLABS, _D, _H, _W).astype(_BF16)
    M = _haar_matrix()
    Mpn = np.ascontiguousarray(np.concatenate([M, -M], axis=1)).astype(_BF16)
    in_maps = [
        {
            "x": np.ascontiguousarray(
                xf[i * _SLABS_PER_CORE : (i + 1) * _SLABS_PER_CORE]
            ),
            "hm": Mpn,
        }
        for i in range(_NCORES)
    ]
    res = run_bass_kernel_spmd(
        _get_nc(), in_maps, core_ids=list(range(_NCORES)), trace=trace, **spmd_kwargs
    )
    outs = [r["y"] for r in res.results]  # each (8, 4, 32, 64, 64) bf16
    full = np.concatenate(outs, axis=1).astype(np.float32)  # (8, 32, 32, 64, 64)
    full = full.reshape(8, _B, _C, _D // 2, _H // 2, _W // 2)
    return full, res


def kernel(**inputs):
    full, _ = _run(inputs["x"])
    return tuple(full[i] for i in range(8))
